# revision 1
# baseline (speedup 1.0000x reference)
# Trainium2 Bass kernel for nn_BDHBlock (dense transformer block).
#
# Strategy (8 NeuronCores, one shared SPMD program):
#   - Token-parallel for all token-local stages: core c owns flat tokens
#     [512c, 512c+512) of x.reshape(4096, 1024). LayerNorms, the masked
#     sparse linear, QKV / output projections and the FFN are computed
#     locally with replicated (host pre-transposed) weights.
#   - Attention is head-parallel: an AllToAll reshards q/k/v from
#     token-sharded to head-sharded (2 heads x full 4096-token sequence per
#     core), each core runs exact-causal relu attention for its 2 heads,
#     and a second AllToAll reshards the context back to token-sharded.
#     This keeps the program identical on every core (static loops).
#   - Matmul dtypes: float32r (full-rate fp32) for all weight-stationary
#     linears; fp16 for attention and ff2 (w2 cast on-chip after a f32 load).
import numpy as np

import concourse.bass as bass
import concourse.mybir as mybir
import concourse.tile as tile
from concourse import bacc
from concourse.masks import make_identity

B, S, H, NH = 2, 2048, 1024, 16
D = H // NH            # 64
FF = 4 * H             # 4096
NC = 8                 # cores
T = B * S // NC        # 512 tokens per core
TT = T // 128          # 4 token tiles
KT = H // 128          # 8 feature tiles
HPC = 2                # heads per core
F32, F32R, F16 = mybir.dt.float32, mybir.dt.float32r, mybir.dt.float16
ADD, SUB, MUL, MAX = (mybir.AluOpType.add, mybir.AluOpType.subtract,
                      mybir.AluOpType.mult, mybir.AluOpType.max)
AF = mybir.ActivationFunctionType
RG = [list(range(NC))]
EPS = 1e-5

_CACHE = {}


def _r(ap):
    return ap.bitcast(F32R)


def _build():
    nc = bacc.Bacc("TRN2", target_bir_lowering=False, debug=False,
                   num_devices=NC)

    # ---------------- I/O ----------------
    def inp(name, shape, dtype=F32):
        return nc.dram_tensor(name, list(shape), dtype, kind="ExternalInput")

    x_io = inp("x_c", (T, H))
    sfwT_io = inp("sfwT", (H, H))
    maskT_io = inp("maskT", (H, H))
    wT_io = {k: inp(k, (H, H)) for k in ("wqT", "wkT", "wvT", "woT")}
    w1T_io = inp("w1T", (H, FF))
    w2T_io = inp("w2T", (FF, H))
    b_io = {k: inp(k, (H,)) for k in ("sf_b", "bq", "bk", "bv", "bo", "ff2_b")}
    ff1b_io = inp("ff1_b", (FF,))
    gb_io = {k: inp(k, (H,)) for k in ("g1", "b1", "g2", "b2", "g3", "b3")}
    tri_io = inp("tri", (2, 128, 256))           # fp32 diag masks
    bqk_col_io = inp("bqk_col", (128, 2 * KT))   # [p, 2*kt]: bq/bk per-partition cols
    ff1b_col_io = inp("ff1b_col", (128, FF // 128))
    out_io = nc.dram_tensor("out_c", [T, H], F32, kind="ExternalOutput")

    # internal DRAM for collectives (HBM bounce; out must be Shared)
    SLOT = 128 * T                               # elements per (dest, tensor) slot
    kv_in = nc.dram_tensor("kv_in", [NC, 2, SLOT], F16)
    kv_out = nc.dram_tensor("kv_out", [NC, 2, SLOT], F16)
    q_in = nc.dram_tensor("q_in", [NC, SLOT], F16)
    q_out = nc.dram_tensor("q_out", [NC, SLOT], F16)
    cc_in = nc.dram_tensor("cc_in", [NC, SLOT], F16)
    cc_out = nc.dram_tensor("cc_out", [NC, SLOT], F16)

    from contextlib import ExitStack
    with tile.TileContext(nc) as tc, ExitStack() as es:
        # ---------------- pools ----------------
        const = es.enter_context(tc.tile_pool(name="const", bufs=1))
        persist = es.enter_context(tc.tile_pool(name="persist", bufs=1))
        wpool = es.enter_context(tc.tile_pool(name="wpool", bufs=6))  # f32 [128,512]
        wpool16 = es.enter_context(tc.tile_pool(name="wpool16", bufs=4))  # f16 weight tiles
        sc_pool = es.enter_context(tc.tile_pool(name="scratch", bufs=3))
        small = es.enter_context(tc.tile_pool(name="small", bufs=8))
        pacc = es.enter_context(tc.tile_pool(name="pacc", bufs=1, space="PSUM"))  # 4 acc tags = 4 banks
        pmix = es.enter_context(tc.tile_pool(name="pmix", bufs=4, space="PSUM"))  # shared rotating tag = 3 banks

        ident = const.tile([128, 128], F32)
        make_identity(nc, ident)
        tri = const.tile([128, 2, 256], F32)
        nc.sync.dma_start(out=tri[:], in_=tri_io.ap().rearrange("a p q -> p a q"))
        bqk_col = const.tile([128, 2 * KT], F32)
        nc.sync.dma_start(out=bqk_col[:], in_=bqk_col_io.ap())
        ff1b_col = const.tile([128, FF // 128], F32)
        nc.sync.dma_start(out=ff1b_col[:], in_=ff1b_col_io.ap())
        eps_col = const.tile([128, 1], F32)
        nc.vector.memset(eps_col[:], EPS)
        ones64 = const.tile([1, 64], F32)
        nc.vector.memset(ones64[:], 1.0)

        _round = [0]
        def acc_tiles():
            r = _round[0]; _round[0] += 1
            if r % 2 == 0:
                return [pacc.tile([128, 512], F32, tag=f"acc{t}", name=f"acc{t}")
                        for t in range(4)]
            return [pmix.tile([128, 512], F32, tag="pmix", name=f"accp{t}")
                    for t in range(4)]

        # residual stream, token-major [128, tt, H]
        x_sb = persist.tile([128, TT, H], F32)
        nc.sync.dma_start(out=x_sb[:], in_=x_io.ap().rearrange("(tt p) h -> p tt h", p=128))

        # slot-sharing tags: bigA = lnT (16K) then h (32K); bigB = qT (8K) then
        # ctxT (16K); bigC = kT then ctxo (8K); bigD = v then ln3T (8K)
        lnT_sb = persist.tile([128, KT, T], F32R, tag="bigA")
        qT_sb = persist.tile([128, KT, T], F16, tag="bigB")
        kT_sb = persist.tile([128, KT, T], F16, tag="bigC")
        v_sb = persist.tile([128, TT, H], F16, tag="bigD")
        g_bc = persist.tile([128, H], F32)              # gamma broadcast scratch
        beta_bc = persist.tile([128, H], F32)           # beta broadcast scratch
        bias_bc = persist.tile([128, H], F32)           # free-dim bias broadcast scratch

        row_pool = es.enter_context(tc.tile_pool(name="rows", bufs=1))

        def bcast_row(dst, src_dram, n):
            """Broadcast a [n] DRAM row across 128 partitions via zero-stride DMA."""
            src = src_dram.ap().unsqueeze(0).partition_broadcast(128).squeeze(1)
            nc.gpsimd.dma_start(out=dst[:, :n], in_=src)

        # ---------------- layernorm (token-major) + transpose ----------------
        def layer_norm_t(g_name, b_name, dst):
            """LN over x_sb tokens; writes transposed output into dst [128, kt, T]."""
            bcast_row(g_bc, gb_io[g_name], H)
            bcast_row(beta_bc, gb_io[b_name], H)
            for tt in range(TT):
                xt = x_sb[:, tt, :]
                sums = small.tile([128, 1], F32, tag="s0")
                sumsq = small.tile([128, 1], F32, tag="s1")
                lt = sc_pool.tile([128, H], F32, tag="lnt")
                nc.vector.reduce_sum(sums[:], xt, axis=mybir.AxisListType.X)
                nc.scalar.activation(lt[:], xt, AF.Square, accum_out=sumsq[:])
                mu = small.tile([128, 1], F32, tag="s2")
                var = small.tile([128, 1], F32, tag="s3")
                rstd = small.tile([128, 1], F32, tag="s4")
                nc.vector.tensor_scalar_mul(mu[:], sums[:], 1.0 / H)
                nc.vector.tensor_scalar_mul(var[:], sumsq[:], 1.0 / H)
                nc.vector.tensor_tensor(rstd[:], mu[:], mu[:], MUL)
                nc.vector.tensor_tensor(var[:], var[:], rstd[:], SUB)
                nc.scalar.activation(rstd[:], var[:], AF.Sqrt, bias=eps_col[:])
                nc.vector.reciprocal(rstd[:], rstd[:])
                nc.vector.tensor_scalar(lt[:], xt, mu[:], rstd[:], op0=SUB, op1=MUL)
                nc.any.tensor_mul(lt[:], lt[:], g_bc[:])
                nc.any.tensor_add(lt[:], lt[:], beta_bc[:])
                for kt in range(KT):
                    pt = pmix.tile([128, 512], F32, tag="pmix", name="pt")
                    nc.tensor.transpose(pt[:, :128], lt[:, bass.ts(kt, 128)], ident[:])
                    nc.any.tensor_copy(dst[:, kt, bass.ts(tt, 128)], pt[:, :128])

        # =====================================================================
        # Stage 1: x += LN1(x) @ (sf_w * mask).T + sf_b
        # =====================================================================
        layer_norm_t("g1", "b1", lnT_sb)
        bcast_row(bias_bc, b_io["sf_b"], H)
        for nch in range(2):
            ps = acc_tiles()
            for kt in range(KT):
                wt = wpool.tile([128, 512], F32R, tag="wa")
                mt = wpool.tile([128, 512], F32, tag="wb")
                nc.sync.dma_start(out=wt[:], in_=_r(sfwT_io.ap()[bass.ts(kt, 128), bass.ts(nch, 512)]))
                nc.sync.dma_start(out=mt[:], in_=maskT_io.ap()[bass.ts(kt, 128), bass.ts(nch, 512)])
                nc.any.tensor_mul(wt[:], wt[:], mt[:])
                for tt in range(TT):
                    nc.tensor.matmul(ps[tt][:], lnT_sb[:, kt, bass.ts(tt, 128)],
                                     wt[:], start=(kt == 0), stop=(kt == KT - 1))
            for tt in range(TT):
                xsl = x_sb[:, tt, bass.ts(nch, 512)]
                tmp = sc_pool.tile([128, 512], F32, tag="ev")
                nc.any.tensor_add(tmp[:], ps[tt][:], bias_bc[:, bass.ts(nch, 512)])
                nc.any.tensor_add(xsl, xsl, tmp[:])

        # =====================================================================
        # Stage 2: LN2 + QKV
        # =====================================================================
        layer_norm_t("g2", "b2", lnT_sb)
        # q/k: feature-major out [n 128, t 512]; scale q/k by 1/sqrt(sqrt(D)) each
        # so scores come out pre-scaled by 1/sqrt(D).
        qsc = 1.0 / float(np.sqrt(np.sqrt(D)))
        for wio, dst, bcol in [("wkT", kT_sb, 1)]:
            for nh in range(2):
                ps = acc_tiles()
                for kt in range(KT):
                    wt = wpool.tile([128, 512], F32R, tag="wa")
                    nc.sync.dma_start(out=wt[:], in_=_r(wT_io[wio].ap()[bass.ts(kt, 128), bass.ts(nh, 512)]))
                    for n4 in range(4):
                        nc.tensor.matmul(ps[n4][:], wt[:, bass.ts(n4, 128)], lnT_sb[:, kt, :],
                                         start=(kt == 0), stop=(kt == KT - 1))
                for n4 in range(4):
                    nt = nh * 4 + n4
                    nc.any.tensor_scalar(dst[:, nt, :], ps[n4][:],
                                         bqk_col[:, bcol * KT + nt: bcol * KT + nt + 1],
                                         qsc, op0=ADD, op1=MUL)
        # v: token-major out [t 128, n 512]
        bcast_row(bias_bc, b_io["bv"], H)
        for nch in range(2):
            ps = acc_tiles()
            for kt in range(KT):
                wt = wpool.tile([128, 512], F32R, tag="wa")
                nc.sync.dma_start(out=wt[:], in_=_r(wT_io["wvT"].ap()[bass.ts(kt, 128), bass.ts(nch, 512)]))
                for tt in range(TT):
                    nc.tensor.matmul(ps[tt][:], lnT_sb[:, kt, bass.ts(tt, 128)],
                                     wt[:], start=(kt == 0), stop=(kt == KT - 1))
            for tt in range(TT):
                nc.any.tensor_add(v_sb[:, tt, bass.ts(nch, 512)], ps[tt][:],
                                  bias_bc[:, bass.ts(nch, 512)])

        # =====================================================================
        # A2A #1a: k/v exchange (overlaps with q production below)
        # =====================================================================
        for j in range(NC):
            nc.sync.dma_start(out=kv_in.ap()[j, 0].rearrange("(p t) -> p t", p=128),
                              in_=kT_sb[:, j, :])
            nc.sync.dma_start(out=kv_in.ap()[j, 1].rearrange("(p tt f) -> p tt f", p=128, tt=TT),
                              in_=v_sb[:, :, bass.ts(j, 128)])
        nc.gpsimd.collective_compute(
            "AllToAll", mybir.AluOpType.bypass, replica_groups=RG,
            ins=[kv_in.ap().opt()], outs=[kv_out.ap().opt()])
        # q production (overlaps the kv A2A)
        for wio, dst, bcol in [("wqT", qT_sb, 0)]:
            for nh in range(2):
                ps = acc_tiles()
                for kt in range(KT):
                    wt = wpool.tile([128, 512], F32R, tag="wa")
                    nc.sync.dma_start(out=wt[:], in_=_r(wT_io[wio].ap()[bass.ts(kt, 128), bass.ts(nh, 512)]))
                    for n4 in range(4):
                        nc.tensor.matmul(ps[n4][:], wt[:, bass.ts(n4, 128)], lnT_sb[:, kt, :],
                                         start=(kt == 0), stop=(kt == KT - 1))
                for n4 in range(4):
                    nt = nh * 4 + n4
                    nc.any.tensor_scalar(dst[:, nt, :], ps[n4][:],
                                         bqk_col[:, bcol * KT + nt: bcol * KT + nt + 1],
                                         qsc, op0=ADD, op1=MUL)
        for j in range(NC):
            nc.sync.dma_start(out=q_in.ap()[j].rearrange("(p t) -> p t", p=128),
                              in_=qT_sb[:, j, :])
        nc.gpsimd.collective_compute(
            "AllToAll", mybir.AluOpType.bypass, replica_groups=RG,
            ins=[q_in.ap().opt()], outs=[q_out.ap().opt()])

        # =====================================================================
        # Attention: 2 heads, full sequence, exact causal
        # =====================================================================
        ctxT_sb = persist.tile([128, B, S], F16, tag="bigB", name="ctxT_sb")
        att_pool = es.enter_context(tc.tile_pool(name="attp", bufs=4))
        qk_pool = es.enter_context(tc.tile_pool(name="qkp", bufs=2))
        vb_pool = es.enter_context(tc.tile_pool(name="vbp", bufs=2))
        rr_pool = es.enter_context(tc.tile_pool(name="rrp", bufs=2))
        SKT = S // 128   # 16 kk tiles per batch
        for b in range(B):
            # v for this batch: [128, src(4), tt(4), h(2), 65] fp16 (65th col = 1)
            vb = vb_pool.tile([128, 4, TT, HPC, D + 1], F16, tag="vb")
            for i in range(4):
                src = 4 * b + i
                nc.sync.dma_start(
                    out=vb[:, i, :, :, 0:D],
                    in_=kv_out.ap()[src, 1].rearrange("(p tt h d) -> p tt h d",
                                                       p=128, tt=TT, h=HPC))
            nc.vector.memset(vb[:, :, :, :, D:D + 1], 1.0)
            for h in range(HPC):
                qa = qk_pool.tile([64, 4, T], F16, tag="qa")
                ka = qk_pool.tile([64, 4, T], F16, tag="ka")
                for i in range(4):
                    src = 4 * b + i
                    nc.sync.dma_start(
                        out=qa[:, i, :],
                        in_=q_out.ap()[src].rearrange("(p t) -> p t", p=128)[bass.ts(h, 64), :])
                    nc.sync.dma_start(
                        out=ka[:, i, :],
                        in_=kv_out.ap()[src, 0].rearrange("(p t) -> p t", p=128)[bass.ts(h, 64), :])
                qf = qa[:].rearrange("p a t -> p (a t)")
                kf = ka[:].rearrange("p a t -> p (a t)")
                for qp in range(S // 256):
                    nkt = 2 * qp + 2
                    cx = pmix.tile([65, 256], F32, tag="pmix", name="cx")
                    for kt in range(nkt):
                        if kt % 2 == 0:
                            sc = pacc.tile([128, 512], F32, tag=f"acc{kt % 4}", name="sc")
                            sc = sc[:, :256]
                        else:
                            sc = pmix.tile([128, 256], F32, tag="pmix", name="sc")
                        att = att_pool.tile([128, 256], F16, tag="att")
                        nc.tensor.matmul(sc[:], kf[:, bass.ts(kt, 128)],
                                         qf[:, bass.ts(qp, 256)], start=True, stop=True)
                        if kt < 2 * qp:
                            nc.any.tensor_scalar_max(att[:], sc[:], 0.0)
                        else:  # diagonal tiles: mask then relu
                            nc.any.tensor_mul(att[:], sc[:], tri[:, kt - 2 * qp, :])
                            nc.any.tensor_scalar_max(att[:], att[:], 0.0)
                        nc.tensor.matmul(cx[:], vb[:, kt // TT, kt % TT, h, :],
                                         att[:], start=(kt == 0), stop=(kt == nkt - 1))
                    # normalize: ctxT[d, q] * 1/(rowsum[q] + 1e-9)
                    rs = rr_pool.tile([1, 256], F32, tag="rs")
                    rb = rr_pool.tile([64, 256], F32, tag="rb")
                    nc.vector.tensor_scalar_add(rs[:], cx[64:65, :], 1e-9)
                    nc.vector.reciprocal(rs[:], rs[:])
                    rbp = pmix.tile([64, 256], F32, tag="pmix", name="rbp")
                    nc.tensor.matmul(rbp[:], ones64[:1, :], rs[:1, :], start=True, stop=True)
                    nc.vector.tensor_copy(rb[:], rbp[:])
                    nc.vector.tensor_tensor(
                        ctxT_sb[bass.ts(h, 64), b, bass.ts(qp, 256)],
                        cx[0:64, :], rb[:], MUL)

        # =====================================================================
        # A2A #2: head-sharded ctx -> token-sharded
        # =====================================================================
        for j in range(NC):
            nc.sync.dma_start(out=cc_in.ap()[j].rearrange("(p t) -> p t", p=128),
                              in_=ctxT_sb[:, :, :].rearrange("p b s -> p (b s)")[:, bass.ds(j * T, T)])
        nc.gpsimd.collective_compute(
            "AllToAll", mybir.AluOpType.bypass, replica_groups=RG,
            ins=[cc_in.ap().opt()], outs=[cc_out.ap().opt()])
        ctxo_sb = persist.tile([128, KT, T], F16, tag="bigC", name="ctxo_sb")
        for j in range(NC):
            nc.sync.dma_start(out=ctxo_sb[:, j, :],
                              in_=cc_out.ap()[j].rearrange("(p t) -> p t", p=128))
        ctxo32 = persist.tile([128, KT, T], F32R, tag="bigB", name="ctxo32")
        for j in range(NC):
            nc.any.tensor_copy(ctxo32[:, j, :], ctxo_sb[:, j, :])

        # =====================================================================
        # out-proj: x += ctx @ wo.T + bo  (fp16)
        # =====================================================================
        bcast_row(bias_bc, b_io["bo"], H)
        for nch in range(2):
            ps = acc_tiles()
            for kt in range(KT):
                wt = wpool.tile([128, 512], F32R, tag="wa")
                nc.sync.dma_start(out=wt[:], in_=_r(wT_io["woT"].ap()[bass.ts(kt, 128), bass.ts(nch, 512)]))
                for tt in range(TT):
                    nc.tensor.matmul(ps[tt][:], ctxo32[:, kt, bass.ts(tt, 128)],
                                     wt[:], start=(kt == 0), stop=(kt == KT - 1))
            for tt in range(TT):
                xsl = x_sb[:, tt, bass.ts(nch, 512)]
                tmp = sc_pool.tile([128, 512], F32, tag="ev")
                nc.any.tensor_add(tmp[:], ps[tt][:], bias_bc[:, bass.ts(nch, 512)])
                nc.any.tensor_add(xsl, xsl, tmp[:])

        # =====================================================================
        # FFN (fp16): x += relu(LN3(x) @ w1.T + b1f) @ w2.T + b2f
        # =====================================================================
        ln3T_sb = persist.tile([128, KT, T], F32R, tag="bigD", name="ln3T_sb")
        layer_norm_t("g3", "b3", ln3T_sb)
        h_sb = persist.tile([128, FF // 128, T], F16, tag="bigA", name="h_sb")
        NFT = FF // 128  # 32
        for nh in range(NFT // 4):
            ps = acc_tiles()
            for kt in range(KT):
                wt = wpool.tile([128, 512], F32R, tag="wa")
                nc.sync.dma_start(out=wt[:], in_=_r(w1T_io.ap()[bass.ts(kt, 128), bass.ts(nh, 512)]))
                for n4 in range(4):
                    nc.tensor.matmul(ps[n4][:], wt[:, bass.ts(n4, 128)], ln3T_sb[:, kt, :],
                                     start=(kt == 0), stop=(kt == KT - 1))
            for n4 in range(4):
                nt = nh * 4 + n4
                nc.scalar.activation(h_sb[:, nt, :], ps[n4][:], AF.Relu,
                                     bias=ff1b_col[:, nt:nt + 1])
        bcast_row(bias_bc, b_io["ff2_b"], H)
        for nch in range(2):
            ps = acc_tiles()
            for kt in range(NFT):
                wf = wpool.tile([128, 512], F32, tag="wb")
                nc.sync.dma_start(out=wf[:], in_=w2T_io.ap()[bass.ts(kt, 128), bass.ts(nch, 512)])
                wt = wpool16.tile([128, 512], F16, tag="w16")
                nc.any.tensor_copy(wt[:], wf[:])
                for tt in range(TT):
                    nc.tensor.matmul(ps[tt][:], h_sb[:, kt, bass.ts(tt, 128)],
                                     wt[:], start=(kt == 0), stop=(kt == NFT - 1))
            for tt in range(TT):
                xsl = x_sb[:, tt, bass.ts(nch, 512)]
                tmp = sc_pool.tile([128, 512], F32, tag="ev")
                nc.any.tensor_add(tmp[:], ps[tt][:], bias_bc[:, bass.ts(nch, 512)])
                nc.any.tensor_add(xsl, xsl, tmp[:])

        # final output
        nc.sync.dma_start(out=out_io.ap().rearrange("(tt p) h -> p tt h", p=128),
                          in_=x_sb[:])

    nc.compile()
    return nc


def _prep_shared(inputs):
    f = lambda a: np.ascontiguousarray(np.asarray(a, np.float32))
    sh = {
        "sfwT": f(inputs["sf_w"]).T.copy(),
        "maskT": f(inputs["mask"]).T.copy(),
        "wqT": f(inputs["wq"]).T.copy(),
        "wkT": f(inputs["wk"]).T.copy(),
        "wvT": f(inputs["wv"]).T.copy(),
        "woT": f(inputs["wo"]).T.copy(),
        "w1T": f(inputs["ff1_w"]).T.copy(),
        "w2T": f(inputs["ff2_w"]).T.copy(),
        "ff1_b": f(inputs["ff1_b"]),
    }
    for k in ("sf_b", "bq", "bk", "bv", "bo"):
        sh[k] = f(inputs[k])
    sh["ff2_b"] = f(inputs["ff2_b"])
    for k in ("g1", "b1", "g2", "b2", "g3", "b3"):
        sh[k] = f(inputs[k])
    # diag masks: tri[0] = [tril.T | ones], tri[1] = [zeros | tril.T]
    tri = np.zeros((2, 128, 256), np.float32)
    tl = np.tril(np.ones((128, 128), np.float32)).T  # valid: kk(row) <= q(col)
    tri[0, :, :128] = tl
    tri[0, :, 128:] = 1.0
    tri[1, :, 128:] = tl
    sh["tri"] = tri
    sh["bqk_col"] = np.stack([sh["bq"], sh["bk"]]).reshape(2 * KT, 128).T.copy().reshape(128, 2 * KT)
    sh["ff1b_col"] = sh["ff1_b"].reshape(FF // 128, 128).T.copy()
    return sh


def kernel(**inputs) -> np.ndarray:
    from concourse.bass_utils import run_bass_kernel_spmd

    if "nc" not in _CACHE:
        _CACHE["nc"] = _build()
    nc = _CACHE["nc"]

    sh = _prep_shared(inputs)
    x = np.ascontiguousarray(np.asarray(inputs["x"], np.float32)).reshape(B * S, H)
    in_maps = []
    for c in range(NC):
        m = dict(sh)
        m["x_c"] = np.ascontiguousarray(x[c * T:(c + 1) * T])
        in_maps.append(m)

    res = run_bass_kernel_spmd(nc, in_maps, core_ids=list(range(NC)))
    out = np.concatenate([res.results[c]["out_c"] for c in range(NC)], axis=0)
    return out.reshape(B, S, H).astype(np.float32)



# revision 11
# speedup vs baseline: 1.3334x; 1.3334x over previous
# Trainium2 Bass kernel for nn_BDHBlock (dense transformer block).
#
# Strategy (8 NeuronCores, one shared SPMD program):
#   - Token-parallel for token-local stages: core c owns flat tokens
#     [512c, 512c+512). LayerNorm affine (g,b) is folded into the following
#     weights on the host, so on-chip LN is a pure normalize. All weights are
#     uploaded fp16 (pre-transposed, pre-shuffled), halving HBM traffic.
#   - Attention is head-parallel: core c owns global heads {2c, 2c+1} over the
#     full 4096-token sequence. q/k/v are exchanged with two AllToAlls split
#     by head parity (lo/hi) so the second exchange and the ctx return ride
#     under attention compute. Attention uses 512-wide q blocks with exact
#     causal masking; score->relu->ctx stages are decoupled across psum banks.
#   - fp16 matmuls everywhere (psum accumulate fp32); bias rows are seeded
#     into psum before accumulation chains to keep vector-engine work low.
import numpy as np

import concourse.bass as bass
import concourse.mybir as mybir
import concourse.tile as tile
from concourse import bacc
from concourse.masks import make_identity

B, S, H, NH = 2, 2048, 1024, 16
D = H // NH            # 64
FF = 4 * H             # 4096
NC = 8                 # cores
T = B * S // NC        # 512 tokens per core
TT = T // 128          # 4 token tiles
KT = H // 128          # 8 feature tiles
NFT = FF // 128        # 32 ffn tiles
F32, F16 = mybir.dt.float32, mybir.dt.float16
ADD, SUB, MUL, MAX = (mybir.AluOpType.add, mybir.AluOpType.subtract,
                      mybir.AluOpType.mult, mybir.AluOpType.max)
AF = mybir.ActivationFunctionType
RG = [list(range(NC))]
EPS = 1e-5

_CACHE = {}


def _build():
    nc = bacc.Bacc("TRN2", target_bir_lowering=False, debug=False,
                   num_devices=NC)

    # ---------------- I/O ----------------
    def inp(name, shape, dtype=F32):
        return nc.dram_tensor(name, list(shape), dtype, kind="ExternalInput")

    x_io = inp("x_c", (T, H))
    sfwT_io = inp("sfwT", (H, H), F16)
    wqT_io = inp("wqT", (H, H), F16)     # col-shuffled (parity, dest, d)
    wkT_io = inp("wkT", (H, H), F16)     # col-shuffled
    wvT_io = inp("wvT", (H, H), F16)     # col-shuffled
    woT_io = inp("woT", (H, H), F16)
    w1T_io = inp("w1T", (H, FF), F16)
    w2T_io = inp("w2T", (FF, H), F16)
    brow_io = inp("brow", (4, H))        # rows: sb1, bv_shuf, bo, ff2_b
    bcol_io = inp("bcol", (128, 2 * KT))  # [p, nt]: bq_shuf, bk_shuf cols
    ff1b_col_io = inp("ff1b_col", (128, NFT))
    tri_io = inp("tri", (4, 128, 512))   # causal diag masks (f32)
    out_io = nc.dram_tensor("out_c", [T, H], F32, kind="ExternalOutput")

    # internal DRAM for collectives (HBM bounce)
    QSL = 64 * T                          # 32768: one head x 512 tokens
    a2a_in = [nc.dram_tensor(f"a2a{p}_in", [NC, 3, QSL], F16) for p in (0, 1)]
    a2a_out = [nc.dram_tensor(f"a2a{p}_out", [NC, 3, QSL], F16) for p in (0, 1)]
    cc_in = [nc.dram_tensor(f"cc{p}_in", [NC, QSL], F16) for p in (0, 1)]
    cc_out = [nc.dram_tensor(f"cc{p}_out", [NC, QSL], F16) for p in (0, 1)]

    from contextlib import ExitStack
    with tile.TileContext(nc) as tc, ExitStack() as es:
        # ---------------- pools ----------------
        const = es.enter_context(tc.tile_pool(name="const", bufs=1))
        persist = es.enter_context(tc.tile_pool(name="persist", bufs=1))
        wpool = es.enter_context(tc.tile_pool(name="wpool", bufs=6))
        att_pool = es.enter_context(tc.tile_pool(name="attp", bufs=6))
        sc_pool = es.enter_context(tc.tile_pool(name="scratch", bufs=2))
        small = es.enter_context(tc.tile_pool(name="small", bufs=4))
        rr_pool = es.enter_context(tc.tile_pool(name="rrp", bufs=2))
        pacc = es.enter_context(tc.tile_pool(name="pacc", bufs=1, space="PSUM"))
        pcx = es.enter_context(tc.tile_pool(name="pcx", bufs=2, space="PSUM"))
        pmix = es.enter_context(tc.tile_pool(name="pmix", bufs=2, space="PSUM"))

        ident16 = const.tile([128, 128], F16)
        make_identity(nc, ident16)
        tri = const.tile([128, 4, 512], F32)
        nc.sync.dma_start(out=tri[:], in_=tri_io.ap().rearrange("a p q -> p a q"))
        bcol = const.tile([128, 2 * KT], F32)
        nc.sync.dma_start(out=bcol[:], in_=bcol_io.ap())
        ff1b_col = const.tile([128, NFT], F32)
        nc.sync.dma_start(out=ff1b_col[:], in_=ff1b_col_io.ap())
        eps_col = const.tile([128, 1], F32)
        nc.vector.memset(eps_col[:], EPS)
        ones64 = const.tile([1, 64], F32)
        nc.vector.memset(ones64[:], 1.0)
        brow = const.tile([128, 4, H], F32)
        nc.sync.dma_start(
            out=brow[:], in_=brow_io.ap().unsqueeze(0).partition_broadcast(128))

        # residual stream, token-major [128, tt, H]
        x_sb = persist.tile([128, TT, H], F32)
        nc.sync.dma_start(out=x_sb[:], in_=x_io.ap().rearrange("(tt p) h -> p tt h", p=128))

        lnT = persist.tile([128, KT, T], F16)       # LN output, feature-major
        kq_local = persist.tile([128, 2, 2, 4, T], F16)  # [p,(k|q),par,m,t]
        vb_local = persist.tile([128, 2, TT, 512], F16)  # [p,par,tt,cols]
        qa = persist.tile([64, 2, B * S], F16)      # [d, head, flat tok]
        ka = persist.tile([64, 2, B * S], F16)
        va = persist.tile([128, 32, 2, D + 1], F16)  # [tokpart, fkt, head, d+1]
        ctxT = persist.tile([64, 2, B * S], F16)
        ctxo = persist.tile([128, KT, T], F16)
        h_sb = persist.tile([128, NFT, T], F16)
        stat4 = persist.tile([128, 6, TT], F32)     # sums/sq/mu/ex2/var/rstd
        junk = persist.tile([128, 2, H], F32)

        nc.vector.memset(va[:], 1.0)                # bakes the ones column

        # ---------------- psum helpers ----------------
        _round = [0]

        def acc_tiles():
            r = _round[0]; _round[0] += 1
            if r % 2 == 0:
                return [pacc.tile([128, 512], F32, tag=f"acc{t}", name=f"acc{t}")
                        for t in range(4)]
            return [pcx.tile([128, 512], F32, tag="cx", name=f"apc{t}") for t in range(2)] + \
                   [pmix.tile([128, 512], F32, tag="pm", name=f"apm{t}") for t in range(2)]

        _eng = [0]

        def alt():
            _eng[0] ^= 1
            return nc.vector if _eng[0] else nc.scalar

        def copy_alt(out, in_):
            _eng[0] ^= 1
            if _eng[0]:
                nc.vector.tensor_copy(out, in_)
            else:
                nc.scalar.copy(out, in_)

        # ---------------- layernorm (pure normalize) -> lnT ----------------
        def layer_norm_t():
            for tt in range(TT):
                xt = x_sb[:, tt, :]
                nc.vector.reduce_sum(stat4[:, 0, tt:tt + 1], xt, axis=mybir.AxisListType.X)
                nc.scalar.activation(junk[:, tt % 2, :], xt, AF.Square,
                                     accum_out=stat4[:, 1, tt:tt + 1])
            nc.vector.tensor_scalar_mul(stat4[:, 2, :], stat4[:, 0, :], 1.0 / H)
            nc.vector.tensor_scalar_mul(stat4[:, 3, :], stat4[:, 1, :], 1.0 / H)
            nc.vector.tensor_tensor(stat4[:, 4, :], stat4[:, 2, :], stat4[:, 2, :], MUL)
            nc.vector.tensor_tensor(stat4[:, 4, :], stat4[:, 3, :], stat4[:, 4, :], SUB)
            nc.scalar.activation(stat4[:, 5, :], stat4[:, 4, :], AF.Sqrt, bias=eps_col[:])
            nc.vector.reciprocal(stat4[:, 5, :], stat4[:, 5, :])
            for tt in range(TT):
                z16 = sc_pool.tile([128, H], F16, tag="z16")
                nc.vector.tensor_scalar(z16[:], x_sb[:, tt, :],
                                        stat4[:, 2, tt:tt + 1], stat4[:, 5, tt:tt + 1],
                                        op0=SUB, op1=MUL)
                for half in range(2):
                    pt = pmix.tile([128, 512], F16, tag="pm", name="pt")
                    for jj in range(4):
                        nc.tensor.transpose(pt[:, bass.ts(jj, 128)],
                                            z16[:, bass.ts(4 * half + jj, 128)], ident16[:])
                    copy_alt(lnT[:, 4 * half:4 * half + 4, bass.ts(tt, 128)],
                             pt[:].rearrange("p (a q) -> p a q", a=4))

        # ---------------- generic token-major linear + residual ----------------
        def linear_residual(wT_io_, brow_idx, src):
            """x += src.T @ w + bias ; src is lnT-style [128, KT, T] fp16."""
            for nch in range(2):
                ps = acc_tiles()
                for tt in range(TT):
                    nc.scalar.activation(ps[tt][:], brow[:, brow_idx, bass.ts(nch, 512)],
                                         AF.Copy)
                for kt in range(KT):
                    wt = wpool.tile([128, 512], F16, tag="w")
                    nc.sync.dma_start(out=wt[:], in_=wT_io_.ap()[bass.ts(kt, 128), bass.ts(nch, 512)])
                    for tt in range(TT):
                        nc.tensor.matmul(ps[tt][:], src[:, kt, bass.ts(tt, 128)],
                                         wt[:], start=False, stop=(kt == KT - 1),
                                         skip_group_check=True)
                for tt in range(TT):
                    nc.vector.tensor_add(x_sb[:, tt, bass.ts(nch, 512)],
                                         x_sb[:, tt, bass.ts(nch, 512)], ps[tt][:])

        # =====================================================================
        # Stage 1: x += LN1(x) @ (sf_w*mask*g1).T + (sfw@b1 + sf_b)
        # =====================================================================
        layer_norm_t()
        linear_residual(sfwT_io, 0, lnT)

        # =====================================================================
        # Stage 2: LN2 + QKV (parity-split) + A2A
        # =====================================================================
        layer_norm_t()

        def kq_proj(wio, which, par):
            """feature-major: kq_local[:, which, par, m, :] = w_chunk.T @ lnT + bias."""
            ps = acc_tiles()
            for kt in range(KT):
                wt = wpool.tile([128, 512], F16, tag="w")
                nc.sync.dma_start(out=wt[:], in_=wio.ap()[bass.ts(kt, 128), bass.ts(par, 512)])
                for m in range(4):
                    nc.tensor.matmul(ps[m][:], wt[:, bass.ts(m, 128)],
                                     lnT[:, kt, :],
                                     start=(kt == 0), stop=(kt == KT - 1))
            for m in range(4):
                nt = par * 4 + m
                bc = bcol[:, which * KT + nt: which * KT + nt + 1]
                e = alt()
                if e is nc.scalar:
                    nc.scalar.activation(kq_local[:, which, par, m, :], ps[m][:],
                                         AF.Identity, bias=bc)
                else:
                    nc.vector.tensor_scalar_add(kq_local[:, which, par, m, :], ps[m][:], bc)

        def v_proj(par):
            ps = acc_tiles()
            for tt in range(TT):
                nc.scalar.activation(ps[tt][:], brow[:, 1, bass.ts(par, 512)], AF.Copy)
            for kt in range(KT):
                wt = wpool.tile([128, 512], F16, tag="w")
                nc.sync.dma_start(out=wt[:], in_=wvT_io.ap()[bass.ts(kt, 128), bass.ts(par, 512)])
                for tt in range(TT):
                    nc.tensor.matmul(ps[tt][:], lnT[:, kt, bass.ts(tt, 128)],
                                     wt[:], start=False, stop=(kt == KT - 1),
                                     skip_group_check=True)
            for tt in range(TT):
                copy_alt(vb_local[:, par, tt, :], ps[tt][:])

        def stage_qkv(par):
            for j in range(NC):
                m, half = j // 2, j % 2
                nc.gpsimd.dma_start(
                    out=a2a_in[par].ap()[j, 0].rearrange("(d t) -> d t", d=64),
                    in_=kq_local[bass.ts(half, 64), 0, par, m, :])
                nc.gpsimd.dma_start(
                    out=a2a_in[par].ap()[j, 1].rearrange("(d t) -> d t", d=64),
                    in_=kq_local[bass.ts(half, 64), 1, par, m, :])
                nc.gpsimd.dma_start(
                    out=a2a_in[par].ap()[j, 2].rearrange("(p tt c) -> p tt c", p=128, tt=TT),
                    in_=vb_local[:, par, :, bass.ds(64 * j, 64)])
            nc.gpsimd.collective_compute(
                "AllToAll", mybir.AluOpType.bypass, replica_groups=RG,
                ins=[a2a_in[par].ap().opt()], outs=[a2a_out[par].ap().opt()])

        def unstage_qkv(par):
            for j in range(NC):
                nc.gpsimd.dma_start(
                    out=ka[:, par, bass.ts(j, T)],
                    in_=a2a_out[par].ap()[j, 0].rearrange("(d t) -> d t", d=64))
                nc.gpsimd.dma_start(
                    out=qa[:, par, bass.ts(j, T)],
                    in_=a2a_out[par].ap()[j, 1].rearrange("(d t) -> d t", d=64))
                nc.gpsimd.dma_start(
                    out=va[:, 4 * j:4 * j + 4, par, 0:D],
                    in_=a2a_out[par].ap()[j, 2].rearrange("(p tt c) -> p tt c", p=128, tt=TT))

        # parity A compute + send
        kq_proj(wkT_io, 0, 0)
        kq_proj(wqT_io, 1, 0)
        v_proj(0)
        stage_qkv(0)
        unstage_qkv(0)
        # parity B compute + send (overlaps A2A-A)
        kq_proj(wkT_io, 0, 1)
        kq_proj(wqT_io, 1, 1)
        v_proj(1)
        stage_qkv(1)
        unstage_qkv(1)

        # =====================================================================
        # Attention: heads {2c (par 0), 2c+1 (par 1)}, exact causal, relu-norm
        # =====================================================================
        def attention_head(h):
            for b in range(B):
                base = b * S
                for qp in range(S // 512):
                    nkt = 4 * qp + 4
                    cx = pcx.tile([65, 512], F32, tag="cx", name="cx")
                    for kt in range(nkt):
                        sc = pacc.tile([128, 512], F32, tag=f"acc{kt % 4}", name="sc")
                        att = att_pool.tile([128, 512], F16, tag="att")
                        nc.tensor.matmul(sc[:], ka[:, h, bass.ds(base + kt * 128, 128)],
                                         qa[:, h, bass.ds(base + qp * 512, 512)],
                                         start=True, stop=True)
                        if kt < 4 * qp:
                            if kt % 2 == 0:
                                nc.scalar.activation(att[:], sc[:], AF.Relu)
                            else:
                                nc.vector.tensor_relu(att[:], sc[:])
                        else:
                            nc.vector.scalar_tensor_tensor(
                                att[:], sc[:], 0.0, tri[:, kt - 4 * qp, :],
                                op0=MAX, op1=MUL)
                        nc.tensor.matmul(cx[:], va[:, b * 16 + kt, h, :], att[:],
                                         start=(kt == 0), stop=(kt == nkt - 1))
                    rs = rr_pool.tile([1, 512], F32, tag="rs")
                    rb = rr_pool.tile([64, 512], F32, tag="rb")
                    nc.vector.tensor_scalar_add(rs[:], cx[64:65, :], 1e-9)
                    nc.vector.reciprocal(rs[:], rs[:])
                    rbp = pmix.tile([64, 512], F32, tag="pm", name="rbp")
                    nc.tensor.matmul(rbp[:], ones64[:1, :], rs[:1, :], start=True, stop=True)
                    nc.vector.tensor_copy(rb[:], rbp[:])
                    nc.vector.tensor_tensor(
                        ctxT[:, h, bass.ds(base + qp * 512, 512)],
                        cx[0:64, :], rb[:], MUL)

        def stage_ctx(par):
            for j in range(NC):
                nc.gpsimd.dma_start(
                    out=cc_in[par].ap()[j].rearrange("(d t) -> d t", d=64),
                    in_=ctxT[:, par, bass.ts(j, T)])
            nc.gpsimd.collective_compute(
                "AllToAll", mybir.AluOpType.bypass, replica_groups=RG,
                ins=[cc_in[par].ap().opt()], outs=[cc_out[par].ap().opt()])

        def unstage_ctx(par):
            for j in range(NC):
                nc.gpsimd.dma_start(
                    out=ctxo[bass.ts(par, 64), j, :],
                    in_=cc_out[par].ap()[j].rearrange("(d t) -> d t", d=64))

        attention_head(0)
        stage_ctx(0)
        attention_head(1)
        unstage_ctx(0)
        stage_ctx(1)
        unstage_ctx(1)

        # =====================================================================
        # out-proj: x += ctx @ wo.T + bo
        # =====================================================================
        linear_residual(woT_io, 2, ctxo)

        # =====================================================================
        # FFN: x += relu(LN3(x) @ w1'.T + b1f) @ w2.T + ff2_b
        # =====================================================================
        layer_norm_t()
        for nh in range(NFT // 4):
            ps = acc_tiles()
            for kt in range(KT):
                wt = wpool.tile([128, 512], F16, tag="w")
                nc.sync.dma_start(out=wt[:], in_=w1T_io.ap()[bass.ts(kt, 128), bass.ts(nh, 512)])
                for m in range(4):
                    nc.tensor.matmul(ps[m][:], wt[:, bass.ts(m, 128)],
                                     lnT[:, kt, :],
                                     start=(kt == 0), stop=(kt == KT - 1))
            for m in range(4):
                nt = nh * 4 + m
                if m % 2 == 0:
                    nc.scalar.activation(h_sb[:, nt, :], ps[m][:], AF.Relu,
                                         bias=ff1b_col[:, nt:nt + 1])
                else:
                    nc.vector.tensor_scalar(h_sb[:, nt, :], ps[m][:],
                                            ff1b_col[:, nt:nt + 1], 0.0,
                                            op0=ADD, op1=MAX)
        for nch in range(2):
            ps = acc_tiles()
            for tt in range(TT):
                nc.scalar.activation(ps[tt][:], brow[:, 3, bass.ts(nch, 512)], AF.Copy)
            for kt in range(NFT):
                wt = wpool.tile([128, 512], F16, tag="w")
                nc.sync.dma_start(out=wt[:], in_=w2T_io.ap()[bass.ts(kt, 128), bass.ts(nch, 512)])
                for tt in range(TT):
                    nc.tensor.matmul(ps[tt][:], h_sb[:, kt, bass.ts(tt, 128)],
                                     wt[:], start=False, stop=(kt == NFT - 1),
                                     skip_group_check=True)
            for tt in range(TT):
                nc.vector.tensor_add(x_sb[:, tt, bass.ts(nch, 512)],
                                     x_sb[:, tt, bass.ts(nch, 512)], ps[tt][:])

        # final output
        nc.sync.dma_start(out=out_io.ap().rearrange("(tt p) h -> p tt h", p=128),
                          in_=x_sb[:])

    nc.compile()
    return nc


def _prep_shared(inputs):
    f = lambda a: np.asarray(a, np.float32)
    f16 = lambda a: np.ascontiguousarray(np.asarray(a, np.float16))
    g1, b1 = f(inputs["g1"]), f(inputs["b1"])
    g2, b2 = f(inputs["g2"]), f(inputs["b2"])
    g3, b3 = f(inputs["g3"]), f(inputs["b3"])

    # stage-1 sparse linear with LN1 affine folded in
    wsf = f(inputs["sf_w"]) * f(inputs["mask"])
    sb1 = wsf @ b1 + f(inputs["sf_b"])
    wsf = wsf * g1[None, :]

    # qkv with LN2 affine folded; q/k pre-scaled by D**-0.25 each
    qsc = float(D) ** -0.25
    wq = f(inputs["wq"]); bq = (wq @ b2 + f(inputs["bq"])) * qsc
    wq = wq * g2[None, :] * qsc
    wk = f(inputs["wk"]); bk = (wk @ b2 + f(inputs["bk"])) * qsc
    wk = wk * g2[None, :] * qsc
    wv = f(inputs["wv"]); bv = wv @ b2 + f(inputs["bv"])
    wv = wv * g2[None, :]

    # ffn with LN3 affine folded
    w1 = f(inputs["ff1_w"]); b1f = w1 @ b3 + f(inputs["ff1_b"])
    w1 = w1 * g3[None, :]

    # column shuffle for head-parity A2A: s = par*512 + j*64 + c  <-  128j+64par+c
    perm = np.empty(H, np.int64)
    for par in range(2):
        for j in range(NC):
            s0 = par * 512 + j * 64
            perm[s0:s0 + 64] = 128 * j + 64 * par + np.arange(64)

    sh = {
        "sfwT": f16(wsf.T),
        "wqT": f16(wq.T[:, perm]),
        "wkT": f16(wk.T[:, perm]),
        "wvT": f16(wv.T[:, perm]),
        "woT": f16(f(inputs["wo"]).T),
        "w1T": f16(w1.T),
        "w2T": f16(f(inputs["ff2_w"]).T),
        "brow": np.ascontiguousarray(np.stack(
            [sb1, bv[perm], f(inputs["bo"]), f(inputs["ff2_b"])]).astype(np.float32)),
        "bcol": np.ascontiguousarray(
            np.stack([bk[perm], bq[perm]]).reshape(2 * KT, 128).T.copy().astype(np.float32)),
        "ff1b_col": np.ascontiguousarray(b1f.reshape(NFT, 128).T.copy().astype(np.float32)),
    }
    # diag masks: tri[i][p, c] = 1 if 128*i + p <= c else 0 (c in 0..512)
    tri = np.zeros((4, 128, 512), np.float32)
    for i in range(4):
        p = np.arange(128)[:, None] + 128 * i
        c = np.arange(512)[None, :]
        tri[i] = (p <= c).astype(np.float32)
    sh["tri"] = tri
    return sh


def kernel(**inputs) -> np.ndarray:
    from concourse.bass_utils import run_bass_kernel_spmd

    if "nc" not in _CACHE:
        _CACHE["nc"] = _build()
    nc = _CACHE["nc"]

    sh = _prep_shared(inputs)
    x = np.ascontiguousarray(np.asarray(inputs["x"], np.float32)).reshape(B * S, H)
    in_maps = []
    for c in range(NC):
        m = dict(sh)
        m["x_c"] = np.ascontiguousarray(x[c * T:(c + 1) * T])
        in_maps.append(m)

    res = run_bass_kernel_spmd(nc, in_maps, core_ids=list(range(NC)))
    out = np.concatenate([res.results[c]["out_c"] for c in range(NC)], axis=0)
    return out.reshape(B, S, H).astype(np.float32)
